# revision 9
# baseline (speedup 1.0000x reference)
"""Trainium2 Bass kernel for nn_RNNModel loss (RNN scan + contrastive sample loss).

Strategy (8 cores, data-parallel, v2):
  - Projected token table P' = emb @ W_ih.T + (b_ih + b_hh), vocab padded to
    32768, sharded 4096 rows/core. Projection tiles run dense before/early in
    the scan; the table AllGather is split into 4 chunks that overlap the scan.
  - Wx for the scan = same projection of the 8192 data tokens, sharded 1024
    rows/core + AllGather (first in the CC queue).
  - RNN scan (128 steps, [64,1024] hidden) replicated on every core. Scan
    matmuls are column-tiled: group A (PSUM partitions 0:64) computes
    h[:, 0:512], group B (partitions 64:128) computes h[:, 512:1024]; the two
    groups execute concurrently in different column strips of the PE array, so
    the 18 matmuls take ~9 slot times. h lives in "split" layout
    h2[128, 512] (rows 0:64 = h cols 0:512, rows 64:128 = h cols 512:1024).
    One tanh per step ([128,512]); positive pairwise term via two DVE
    scalar_tensor_tensor ops; hT rebuilt per step half by PE transpose
    (chunks 0-3) and half by DMA transpose (chunks 4-7).
  - Negative block re-sharded t-strided: core c owns t in {8u + c}. Its
    position tile j covers t in {16j+c, 16j+8+c}, which is complete by scan
    step 16j+15 on every core, so hU matmuls and the per-sample elementwise
    chains interleave INTO the scan instead of running as a serial tail.
  - Host sums per-core partials (pos from core 0; neg from all cores).
"""

import numpy as np
import ml_dtypes
from contextlib import ExitStack

V, H, S, B, NS, NC = 32000, 1024, 128, 64, 10, 8
VP = 32768           # padded vocab
N = S * B            # 8192 positions
VSH = VP // NC       # 4096 table rows per core
PSH = N // NC        # 1024 positions per core
NPT = VSH // 128     # 32 proj tiles per core
NCHUNK = 4           # AllGather chunks (8 tiles each)
TEMP, CLIP_DIST, EPS = 65.0, 0.01, 1e-6

_CACHE = {}


def _build():
    import concourse.bass as bass
    import concourse.tile as tile
    from concourse import bacc, mybir
    from concourse.masks import make_identity

    f32 = mybir.dt.float32
    bf16 = mybir.dt.bfloat16
    i32 = mybir.dt.int32
    AF = mybir.ActivationFunctionType
    OP = mybir.AluOpType

    nc = bacc.Bacc("TRN2", target_bir_lowering=False, debug=False, num_devices=NC)

    # ---- I/O ----
    emb = nc.dram_tensor("emb", [V, H], f32, kind="ExternalInput")
    wihT = nc.dram_tensor("wihT", [H, H], bf16, kind="ExternalInput")
    whhT = nc.dram_tensor("whhT", [H, H], bf16, kind="ExternalInput")
    bias2 = nc.dram_tensor("bias2", [1, H], bf16, kind="ExternalInput")
    wx_idx = nc.dram_tensor("wx_idx", [PSH, 1], i32, kind="ExternalInput")
    ps_idx = nc.dram_tensor("ps_idx", [VSH, 1], i32, kind="ExternalInput")
    samp_idx = nc.dram_tensor("samp_idx", [128, 80], i32, kind="ExternalInput")
    prev_idx = nc.dram_tensor("prev_idx", [128, 1], i32, kind="ExternalInput")
    pos_out = nc.dram_tensor("pos_out", [1, 1], f32, kind="ExternalOutput")
    neg_out = nc.dram_tensor("neg_out", [1, 1], f32, kind="ExternalOutput")

    # ---- internal DRAM ----
    wx_sh = nc.dram_tensor("wx_sh", [PSH, H], bf16)
    wx_all = nc.dram_tensor("wx_all", [N, H], bf16, addr_space="Shared")
    p_sh = [nc.dram_tensor(f"p_sh{k}", [1024, H], bf16) for k in range(NCHUNK)]
    p_all = nc.dram_tensor("p_all", [VP, H], bf16, addr_space="Shared")
    raw_j = [nc.dram_tensor(f"raw{j}", [1024, H], bf16) for j in range(8)]

    groups = [list(range(NC))]

    # ---- static schedule ----
    PRE_TILES = 18                      # proj tiles emitted before the scan
    proj_at_step = {}                   # step -> proj tile index
    for i in range(PRE_TILES, NPT):
        proj_at_step[1 + 2 * (i - PRE_TILES)] = i
    ag_after_tile = {7: 0, 15: 1, 23: 2, 31: 3}

    # negative pair schedule: pair (j, s) earliest step
    GATE = 62
    pairs = sorted(
        [(max(16 * j + 20, GATE), j, s) for j in range(8) for s in range(NS)]
    )
    pair_at_step = {}                   # step -> list of (j, s)
    tail_pairs = []
    cur = GATE
    for e, j, s in pairs:
        t = max(e, cur)
        cur = t + 1
        if t <= S:
            pair_at_step.setdefault(t, []).append((j, s))
        else:
            tail_pairs.append((j, s))

    with tile.TileContext(nc) as tc, ExitStack() as ctx:
        const = ctx.enter_context(tc.tile_pool(name="const", bufs=1))
        io = ctx.enter_context(tc.tile_pool(name="io", bufs=4))
        wk = ctx.enter_context(tc.tile_pool(name="wk", bufs=3))
        hp = ctx.enter_context(tc.tile_pool(name="hp", bufs=3))
        pp_scan = ctx.enter_context(tc.tile_pool(name="pp_scan", bufs=2, space="PSUM"))
        pp_tr = ctx.enter_context(tc.tile_pool(name="pp_tr", bufs=2, space="PSUM"))
        pp_big = ctx.enter_context(tc.tile_pool(name="pp_big", bufs=2, space="PSUM"))

        # ---- constants / weights in SBUF ----
        wihT_sb = const.tile([128, 8 * H], bf16)
        whhT_sb = const.tile([128, 8 * H], bf16)
        for kt in range(8):
            nc.sync.dma_start(wihT_sb[:, kt * H:(kt + 1) * H], wihT[kt * 128:(kt + 1) * 128, :])
            nc.sync.dma_start(whhT_sb[:, kt * H:(kt + 1) * H], whhT[kt * 128:(kt + 1) * 128, :])
        bias2_sb = const.tile([1, H], bf16)
        nc.sync.dma_start(bias2_sb[:], bias2[:, :])
        ones1 = const.tile([1, 128], bf16)
        nc.vector.memset(ones1[:], 1.0)
        I64 = const.tile([64, 64], bf16)
        make_identity(nc, I64[:])
        ones128f = const.tile([128, 1], f32)
        nc.vector.memset(ones128f[:], 1.0)
        pos_acc = const.tile([128, 1], f32)
        nc.vector.memset(pos_acc[:], 0.0)
        eps128 = const.tile([128, 1], f32)
        nc.vector.memset(eps128[:], EPS)
        negmat = const.tile([128, 8], f32)
        dmat = const.tile([128, 80], f32)
        hU_stash = const.tile([128, 8 * H], bf16)
        prev_stash = const.tile([128, 8 * H], bf16)
        sidx_all = const.tile([128, 80], i32)
        nc.sync.dma_start(sidx_all[:], samp_idx[:, :])
        pidx_all = const.tile([128, 1], i32)
        nc.sync.dma_start(pidx_all[:], prev_idx[:, :])

        # ---- projection tile: rows of emb -> rows of (e @ W_ih.T + bias2) ----
        def proj_tile(idx_ap, dst_ap, it, dst_row):
            idx_t = io.tile([128, 1], i32, tag="idx")
            nc.sync.dma_start(idx_t[:], idx_ap[it * 128: (it + 1) * 128, :])
            ew = wk.tile([128, H], f32, tag="ew")
            nc.gpsimd.indirect_dma_start(
                out=ew[:], out_offset=None, in_=emb[:, :],
                in_offset=bass.IndirectOffsetOnAxis(ap=idx_t[:, :1], axis=0))
            ewb = wk.tile([128, H], bf16, tag="ewb")
            nc.vector.tensor_copy(ewb[:], ew[:])
            eT = wk.tile([128, 8 * 128], bf16, tag="eT")
            nc.sync.dma_start_transpose(
                out=eT[:].rearrange("p (k b) -> p k b", b=128),
                in_=ewb[:, :])
            ps = pp_big.tile([128, H], f32, tag="proj_ps")
            for sl in (slice(0, 512), slice(512, 1024)):
                nc.tensor.matmul(ps[:, sl], lhsT=ones1[:1, :],
                                 rhs=bias2_sb[:1, sl], start=True, stop=False,
                                 skip_group_check=True)
            for k in range(8):
                for half in range(2):
                    sl = slice(half * 512, (half + 1) * 512)
                    nc.tensor.matmul(
                        ps[:, sl],
                        lhsT=eT[:, k * 128:(k + 1) * 128],
                        rhs=wihT_sb[:, k * H + half * 512: k * H + (half + 1) * 512],
                        start=False, stop=(k == 7), skip_group_check=True)
            ob = wk.tile([128, H], bf16, tag="ob")
            nc.scalar.copy(ob[:], ps[:])
            nc.sync.dma_start(dst_ap[dst_row: dst_row + 128, :], ob[:])

        # ---- Wx shard (8 tiles) + AllGather (first in CC queue) ----
        for it in range(PSH // 128):
            proj_tile(wx_idx, wx_sh, it, it * 128)
        nc.gpsimd.collective_compute(
            "AllGather", mybir.AluOpType.bypass, replica_groups=groups,
            ins=[wx_sh.ap().opt()], outs=[wx_all.ap().opt()])

        def emit_proj(i):
            proj_tile(ps_idx, p_sh[i // 8], i, (i % 8) * 128)
            if i in ag_after_tile:
                k = ag_after_tile[i]
                nc.gpsimd.collective_compute(
                    "AllGather", mybir.AluOpType.bypass, replica_groups=groups,
                    ins=[p_sh[k].ap().opt()],
                    outs=[p_all[8192 * k: 8192 * (k + 1), :].opt()])

        # ---- pre-scan proj tiles ----
        for i in range(PRE_TILES):
            emit_proj(i)

        # ---- hU tile j: gather prev rows, project through W_hh ----
        def hU_gather(j):
            nc.gpsimd.indirect_dma_start(
                out=prev_stash[:, j * H:(j + 1) * H], out_offset=None,
                in_=raw_j[j][:, :],
                in_offset=bass.IndirectOffsetOnAxis(ap=pidx_all[:, :1], axis=0))
            prevT = wk.tile([128, 8 * 128], bf16, tag="prevT")
            nc.sync.dma_start_transpose(
                out=prevT[:].rearrange("p (k b) -> p k b", b=128),
                in_=prev_stash[:, j * H:(j + 1) * H])
            return prevT

        hU_ps = {}
        hU_prevT = {}

        def hU_mms(j, half):
            ps = hU_ps[j]
            prevT = hU_prevT[j]
            sl = slice(half * 512, (half + 1) * 512)
            for k in range(8):
                nc.tensor.matmul(
                    ps[:, sl],
                    lhsT=prevT[:, k * 128:(k + 1) * 128],
                    rhs=whhT_sb[:, k * H + half * 512: k * H + (half + 1) * 512],
                    start=(k == 0), stop=(k == 7), skip_group_check=True)

        def hU_fin(j):
            nc.scalar.copy(hU_stash[:, j * H:(j + 1) * H], hU_ps[j][:])
            del hU_ps[j], hU_prevT[j]

        # ---- negative pair (j, s) ----
        def emit_pair(j, s, parity):
            col = j * 10 + s
            spw = wk.tile([128, H], bf16, tag="spw")
            nc.gpsimd.indirect_dma_start(
                out=spw[:], out_offset=None, in_=p_all[:, :],
                in_offset=bass.IndirectOffsetOnAxis(ap=sidx_all[:, col:col + 1], axis=0))
            pre = wk.tile([128, H], bf16, tag="pre")
            nc.vector.scalar_tensor_tensor(
                out=pre[:], in0=spw[:], scalar=0.0, in1=hU_stash[:, j * H:(j + 1) * H],
                op0=OP.add, op1=OP.add)
            outt = wk.tile([128, H], bf16, tag="outt")
            nc.scalar.activation(outt[:], pre[:], AF.Tanh)
            d = wk.tile([128, H], bf16, tag="dneg")
            nc.vector.scalar_tensor_tensor(
                out=d[:], in0=prev_stash[:, j * H:(j + 1) * H], scalar=EPS,
                in1=outt[:], op0=OP.add, op1=OP.subtract)
            sqx = wk.tile([128, H], bf16, tag="sqx")
            if parity == 0:
                nc.vector.scalar_tensor_tensor(
                    out=sqx[:], in0=d[:], scalar=0.0, in1=d[:],
                    op0=OP.add, op1=OP.mult, accum_out=dmat[:, col:col + 1])
            else:
                nc.scalar.activation(sqx[:], d[:], AF.Square,
                                     accum_out=dmat[:, col:col + 1])
            if s == NS - 1:
                dc = wk.tile([128, NS], f32, tag="dc")
                nc.vector.tensor_scalar_min(dc[:], dmat[:, j * 10:(j + 1) * 10], CLIP_DIST)
                ex = wk.tile([128, NS], f32, tag="ex")
                se = wk.tile([128, 1], f32, tag="se")
                nc.scalar.activation(ex[:], dc[:], AF.Exp, scale=-1.0, accum_out=se[:])
                nc.scalar.activation(negmat[:, j:j + 1], se[:], AF.Ln,
                                     bias=eps128[:], scale=1.0 / N)

        # ---- scan init ----
        h2_prev = hp.tile([128, 512], bf16, tag="h2")
        nc.vector.memset(h2_prev[:], 0.0)
        hT_prev = hp.tile([128, 512], bf16, tag="hT")
        nc.vector.memset(hT_prev[:], 0.0)
        nc.sync.dma_start(raw_j[0][0:64, 0:512], h2_prev[0:64, :])
        nc.sync.dma_start(raw_j[0][0:64, 512:1024], h2_prev[64:128, :])

        pair_parity = 0

        # ---- scan ----
        for t in range(1, S + 1):
            wx_t = io.tile([64, H], bf16, tag="wx")
            nc.sync.dma_start(wx_t[:], wx_all[(t - 1) * 64: t * 64, :])

            ps = pp_scan.tile([128, 512], f32, tag="scan_ps")
            # identity matmuls inject Wx into both column groups
            nc.tensor.matmul(ps[0:64, :], lhsT=I64[:], rhs=wx_t[:, 0:512],
                             start=True, stop=False, skip_group_check=True)
            nc.tensor.matmul(ps[64:128, :], lhsT=I64[:], rhs=wx_t[:, 512:1024],
                             start=True, stop=False, skip_group_check=True)
            for k in range(8):
                lhs = hT_prev[:, 64 * k: 64 * (k + 1)]
                nc.tensor.matmul(
                    ps[0:64, :], lhsT=lhs,
                    rhs=whhT_sb[:, k * H: k * H + 512],
                    start=False, stop=(k == 7), skip_group_check=True)
                nc.tensor.matmul(
                    ps[64:128, :], lhsT=lhs,
                    rhs=whhT_sb[:, k * H + 512: (k + 1) * H],
                    start=False, stop=(k == 7), skip_group_check=True)

            # proj tile MMs fill the tanh bubble
            if t in proj_at_step:
                emit_proj(proj_at_step[t])

            h2_cur = hp.tile([128, 512], bf16, tag="h2")
            nc.scalar.activation(h2_cur[:], ps[:], AF.Tanh)

            # positive term on split layout
            dtile = wk.tile([128, 512], bf16, tag="dpos")
            nc.vector.scalar_tensor_tensor(
                out=dtile[:], in0=h2_prev[:], scalar=EPS, in1=h2_cur[:],
                op0=OP.add, op1=OP.subtract)
            sqt = wk.tile([128, 512], bf16, tag="sqpos")
            pc = wk.tile([128, 1], f32, tag="pc")
            nc.vector.scalar_tensor_tensor(
                out=sqt[:], in0=dtile[:], scalar=0.0, in1=dtile[:],
                op0=OP.add, op1=OP.mult, accum_out=pc[:])
            nc.vector.tensor_tensor(out=pos_acc[:], in0=pos_acc[:], in1=pc[:], op=OP.add)

            if t < S:
                hT_cur = hp.tile([128, 512], bf16, tag="hT")
                # chunks 0-3 via PE transpose (fills hT-copy bubble on ACT)
                psT = pp_tr.tile([128, 256], bf16, tag="trp")
                for bk in range(4):
                    nc.tensor.transpose(
                        psT[:, 64 * bk: 64 * (bk + 1)],
                        in_=h2_cur[0:64, 128 * bk: 128 * (bk + 1)],
                        identity=I64[:])

                # hU matmuls fill PE time while ACT copies psT
                if t >= 18 and (t - 18) % 16 == 0 and (t - 18) // 16 < 7:
                    hU_mms((t - 18) // 16, 0)
                elif t >= 19 and (t - 19) % 16 == 0 and (t - 19) // 16 < 7:
                    hU_mms((t - 19) // 16, 1)

                nc.scalar.copy(hT_cur[:, 0:256], psT[:])
                # chunks 4-7 via DMA transpose
                nc.sync.dma_start_transpose(
                    out=hT_cur[:, 256:512].rearrange("p (k b) -> p k b", b=64),
                    in_=h2_cur[64:128, :])
            else:
                hT_cur = None

            # store h_t into raw (prev rows) unless t == S
            if t < S:
                jj, rr = t // 16, (t % 16) * 64
                nc.sync.dma_start(raw_j[jj][rr:rr + 64, 0:512], h2_cur[0:64, :])
                nc.sync.dma_start(raw_j[jj][rr:rr + 64, 512:1024], h2_cur[64:128, :])

            # hU gather / cast events
            if t >= 16 and t % 16 == 0 and t // 16 - 1 < 7:
                j = t // 16 - 1
                hU_ps[j] = pp_big.tile([128, H], f32, tag="proj_ps", name=f"hU_ps{j}")
                hU_prevT[j] = hU_gather(j)
            if t >= 19 and (t - 19) % 16 == 0 and (t - 19) // 16 < 7:
                hU_fin((t - 19) // 16)

            # negative pairs
            for (j, s) in pair_at_step.get(t, []):
                emit_pair(j, s, pair_parity)
                pair_parity ^= 1

            h2_prev = h2_cur
            if hT_cur is not None:
                hT_prev = hT_cur

        # ---- tail: hU for j=7 + remaining pairs ----
        j = 7
        hU_ps[j] = pp_big.tile([128, H], f32, tag="proj_ps", name=f"hU_ps{j}")
        hU_prevT[j] = hU_gather(j)
        hU_mms(j, 0)
        hU_mms(j, 1)
        hU_fin(j)
        for (j, s) in tail_pairs:
            emit_pair(j, s, pair_parity)
            pair_parity ^= 1

        # ---- finalize scalars ----
        psn = pp_scan.tile([1, 8], f32, tag="scan_ps")
        nc.tensor.matmul(psn[:], lhsT=ones128f[:, :1], rhs=negmat[:], start=True, stop=True)
        scr = wk.tile([1, 8], f32, tag="scr")
        negsc = wk.tile([1, 1], f32, tag="negsc")
        nc.scalar.activation(scr[:], psn[:], AF.Identity, accum_out=negsc[:])
        nc.sync.dma_start(neg_out[:, :], negsc[:])
        psp = pp_scan.tile([1, 1], f32, tag="scan_ps")
        nc.tensor.matmul(psp[:], lhsT=pos_acc[:], rhs=ones128f[:, :1], start=True, stop=True)
        possc = wk.tile([1, 1], f32, tag="possc")
        nc.scalar.mul(possc[:], psp[:], TEMP / S)
        nc.sync.dma_start(pos_out[:, :], possc[:])

    nc.compile()
    return nc


def _get_nc():
    if "nc" not in _CACHE:
        _CACHE["nc"] = _build()
    return _CACHE["nc"]


def kernel(**inputs):
    from concourse.bass_utils import run_bass_kernel_spmd

    bf = ml_dtypes.bfloat16
    data = np.asarray(inputs["data"]).astype(np.int32)          # [S, B]
    samples = np.asarray(inputs["samples"]).astype(np.int64)    # [NS, N]
    emb_W = np.asarray(inputs["emb_W"], dtype=np.float32)
    W_ih = np.asarray(inputs["W_ih"], dtype=np.float32)
    b_ih = np.asarray(inputs["b_ih"], dtype=np.float32)
    W_hh = np.asarray(inputs["W_hh"], dtype=np.float32)
    b_hh = np.asarray(inputs["b_hh"], dtype=np.float32)

    nc = _get_nc()

    wihT = np.ascontiguousarray(W_ih.T).astype(bf)
    whhT = np.ascontiguousarray(W_hh.T).astype(bf)
    bias2 = (b_ih + b_hh).reshape(1, H).astype(bf)
    data_flat = data.reshape(N)  # t-major

    # token -> p_all row (chunked AllGather layout)
    cc = samples // VSH
    rr = samples % VSH
    prow = (8192 * (rr // 1024) + 1024 * cc + (rr % 1024)).astype(np.int32)  # [NS, N]

    ll = np.arange(128)
    t_local = 8 * (ll // 64)            # [128]
    jj = np.arange(8)

    in_maps = []
    for c in range(NC):
        # t-strided negative-position sharding
        pos = (64 * (16 * jj[:, None] + t_local[None, :] + c)
               + (ll % 64)[None, :])    # [8, 128] global positions
        samp = np.empty((128, 80), dtype=np.int32)
        for j in range(8):
            for s in range(NS):
                samp[:, j * 10 + s] = prow[s, pos[j]]
        prev_local = (64 * (t_local + c) + (ll % 64)).astype(np.int32)

        gidx = np.arange(c * VSH, (c + 1) * VSH, dtype=np.int32)
        gidx[gidx >= V] = 0

        in_maps.append({
            "emb": emb_W,
            "wihT": wihT,
            "whhT": whhT,
            "bias2": bias2,
            "wx_idx": data_flat[c * PSH:(c + 1) * PSH].reshape(PSH, 1).astype(np.int32),
            "ps_idx": gidx.reshape(VSH, 1),
            "samp_idx": samp,
            "prev_idx": prev_local.reshape(128, 1),
        })

    res = run_bass_kernel_spmd(nc, in_maps, core_ids=list(range(NC)))
    _CACHE["last_res"] = res
    pos_v = float(res.results[0]["pos_out"].ravel()[0])
    neg_v = sum(float(r["neg_out"].ravel()[0]) for r in res.results)
    return np.float32(pos_v + neg_v)
